# revision 1
# baseline (speedup 1.0000x reference)
"""Trainium2 Bass kernel: LSTM encoder-decoder (IoT anomaly detector).

Reference semantics (B=256, T=512, I=128, H=256):
  encoder LSTM over x[B,T,I] -> final (h,c); pred_last = sigmoid(h @ lin_W.T + lin_b)
  decoder LSTM run T-1 steps feeding back its own prediction; outputs in
  forward time order [B,T,I].

Sharding: pure data parallelism, batch 256 -> 8 cores x 32.

Per-core layout (b=32 local batch), everything "transposed": gate/hidden
dims on SBUF partitions, batch on the free dim.  gates.T is [1024, 32] in 8
chunks of 128 partitions, stored in one PSUM tile with chunk m at cols
[32m, 32m+32), chunk order [f0 f1 i0 i1 g0 g1 o0 o1] so sigmoid(f,i) is one
ACT op over cols 0:128, tanh(g) over 128:192, sigmoid(o) over 192:256.
Weights are bf16 (fast weight load keeps LDWEIGHTS+MATMUL pairs at ~27 ns);
gate accumulation is fp32 in PSUM; c is fp32, h/pred feed back as bf16
(validated 2e-4 absmax vs the fp32 reference).  Biases enter as one matmul
per chunk: lhsT rows 0/1 = bf16 hi/lo halves of the bias, rhs = e0 (rows
0,1 = 1), so bias precision is ~fp32 at zero extra instructions on the
critical path.  The whole x shard, all weights and the preds output buffer
live in SBUF; one DMA in at start, one DMA out at the end.
"""

import numpy as np
import ml_dtypes

B, T, I, H = 256, 512, 128, 256
NCORES = 8
LB = B // NCORES  # 32 local batch

BF16 = ml_dtypes.bfloat16

_BUILT = {}


def _build(t_steps):
    import concourse.bass as bass
    import concourse.tile as tile
    from concourse import bacc, mybir

    f32 = mybir.dt.float32
    bf16 = mybir.dt.bfloat16
    AF = mybir.ActivationFunctionType
    ALU = mybir.AluOpType

    nc = bacc.Bacc(
        "TRN2", target_bir_lowering=False, debug=False, num_devices=NCORES
    )

    xT_d = nc.dram_tensor("xT", [128, t_steps * LB], bf16, kind="ExternalInput")
    wih_e_d = nc.dram_tensor("wih_e", [128, 8 * 128], bf16, kind="ExternalInput")
    whh_e_d = nc.dram_tensor("whh_e", [128, 16 * 128], bf16, kind="ExternalInput")
    bias_e_d = nc.dram_tensor("bias_e", [128, 8 * 128], bf16, kind="ExternalInput")
    wih_d_d = nc.dram_tensor("wih_d", [128, 8 * 128], bf16, kind="ExternalInput")
    whh_d_d = nc.dram_tensor("whh_d", [128, 16 * 128], bf16, kind="ExternalInput")
    bias_d_d = nc.dram_tensor("bias_d", [128, 8 * 128], bf16, kind="ExternalInput")
    wlin_d = nc.dram_tensor("wlin", [128, 2 * 128], bf16, kind="ExternalInput")
    bias_l_d = nc.dram_tensor("bias_l", [128, 128], bf16, kind="ExternalInput")
    e0_d = nc.dram_tensor("e0", [128, LB], bf16, kind="ExternalInput")
    out_d = nc.dram_tensor("out", [128, t_steps * LB], f32, kind="ExternalOutput")

    with tile.TileContext(nc) as tc:
        from contextlib import ExitStack

        with ExitStack() as ctx:
            const = ctx.enter_context(tc.tile_pool(name="const", bufs=1))
            work = ctx.enter_context(tc.tile_pool(name="work", bufs=2))
            psum = ctx.enter_context(
                tc.tile_pool(name="psum", bufs=2, space="PSUM")
            )
            psum2 = ctx.enter_context(
                tc.tile_pool(name="psum2", bufs=2, space="PSUM")
            )

            def load(dram, shape, dt):
                t = const.tile(shape, dt, tag=dram.name)
                nc.sync.dma_start(out=t[:], in_=dram[:])
                return t

            xT = load(xT_d, [128, t_steps * LB], bf16)
            wih_e = load(wih_e_d, [128, 8 * 128], bf16)
            whh_e = load(whh_e_d, [128, 16 * 128], bf16)
            bias_e = load(bias_e_d, [128, 8 * 128], bf16)
            wih_dd = load(wih_d_d, [128, 8 * 128], bf16)
            whh_dd = load(whh_d_d, [128, 16 * 128], bf16)
            bias_dd = load(bias_d_d, [128, 8 * 128], bf16)
            wlin = load(wlin_d, [128, 2 * 128], bf16)
            bias_l = load(bias_l_d, [128, 128], bf16)
            e0 = load(e0_d, [128, LB], bf16)

            preds = const.tile([128, t_steps * LB], f32, tag="preds")
            c = const.tile([128, 64], f32, tag="c")
            h = const.tile([128, 64], bf16, tag="h")
            predb = const.tile([128, LB], bf16, tag="predb")

            nc.vector.memset(c[:], 0.0)
            nc.vector.memset(h[:], 0.0)

            def cell(rhs_x, wih, whh, bias):
                ps = psum.tile([128, 256], f32, tag="gates")
                for m in range(8):
                    o = ps[:, 32 * m : 32 * m + 32]
                    nc.tensor.matmul(
                        o, bias[:, 128 * m : 128 * (m + 1)], e0[:],
                        start=True, stop=False,
                    )
                    nc.tensor.matmul(
                        o, wih[:, 128 * m : 128 * (m + 1)], rhs_x,
                        start=False, stop=False,
                    )
                    nc.tensor.matmul(
                        o, whh[:, 128 * m : 128 * (m + 1)], h[:, 0:32],
                        start=False, stop=False,
                    )
                    nc.tensor.matmul(
                        o, whh[:, 128 * (8 + m) : 128 * (9 + m)], h[:, 32:64],
                        start=False, stop=True,
                    )
                S = work.tile([128, 128], f32, tag="S")
                nc.scalar.activation(S[:], ps[:, 0:128], AF.Sigmoid)
                G = work.tile([128, 64], f32, tag="G")
                nc.scalar.activation(G[:], ps[:, 128:192], AF.Tanh)
                O = work.tile([128, 64], f32, tag="O")
                nc.scalar.activation(O[:], ps[:, 192:256], AF.Sigmoid)
                fc = work.tile([128, 64], f32, tag="fc")
                nc.vector.tensor_mul(fc[:], S[:, 0:64], c[:])
                u = work.tile([128, 64], f32, tag="u")
                nc.vector.tensor_mul(u[:], S[:, 64:128], G[:])
                nc.vector.tensor_add(c[:], fc[:], u[:])
                Tc = work.tile([128, 64], f32, tag="Tc")
                nc.scalar.activation(Tc[:], c[:], AF.Tanh)
                nc.vector.tensor_mul(h[:], O[:], Tc[:])

            def lin_block(slot):
                ps2 = psum2.tile([128, LB], f32, tag="lin")
                nc.tensor.matmul(ps2[:], bias_l[:], e0[:], start=True, stop=False)
                nc.tensor.matmul(
                    ps2[:], wlin[:, 0:128], h[:, 0:32], start=False, stop=False
                )
                nc.tensor.matmul(
                    ps2[:], wlin[:, 128:256], h[:, 32:64], start=False, stop=True
                )
                sl = preds[:, LB * slot : LB * (slot + 1)]
                nc.scalar.activation(sl, ps2[:], AF.Sigmoid)
                nc.vector.tensor_copy(predb[:], sl)

            for t in range(t_steps):
                cell(xT[:, LB * t : LB * (t + 1)], wih_e, whh_e, bias_e)
            lin_block(t_steps - 1)
            for k in range(t_steps - 1):
                cell(predb[:], wih_dd, whh_dd, bias_dd)
                lin_block(t_steps - 2 - k)

            nc.sync.dma_start(out=out_d[:], in_=preds[:])

    nc.compile()
    return nc


def _get(t_steps):
    if t_steps not in _BUILT:
        _BUILT[t_steps] = _build(t_steps)
    return _BUILT[t_steps]


def _pack_weights(enc_W_ih, enc_W_hh, enc_b_ih, enc_b_hh,
                  dec_W_ih, dec_W_hh, dec_b_ih, dec_b_hh, lin_W, lin_b):
    # chunk order [f0 f1 i0 i1 g0 g1 o0 o1]; torch gate rows are [i f g o].
    # g rows are scaled by 2 (tanh(g) = 2*sigmoid(2g) - 1).
    perm = np.r_[H : 2 * H, 0:H, 2 * H : 3 * H, 3 * H : 4 * H]

    def pack_ih(W):  # [4H, I] -> [128, 8*128] lhsT tiles
        Wp = W[perm].reshape(8, 128, I)
        return np.concatenate([Wp[m].T for m in range(8)], axis=1).astype(BF16)

    def pack_hh(W):  # [4H, H] -> [128, 16*128], tile (k,m) at col 128*(8k+m)
        Wp = W[perm]
        tiles = [
            Wp[128 * m : 128 * (m + 1), 128 * k : 128 * (k + 1)].T
            for k in range(2)
            for m in range(8)
        ]
        return np.concatenate(tiles, axis=1).astype(BF16)

    def pack_bias(b):  # [4H] -> [128, 8*128] hi/lo rows
        bp = b[perm].astype(np.float32)
        out = np.zeros((128, 8 * 128), np.float32)
        for m in range(8):
            chunk = bp[128 * m : 128 * (m + 1)]
            hi = chunk.astype(BF16).astype(np.float32)
            out[0, 128 * m : 128 * (m + 1)] = hi
            out[1, 128 * m : 128 * (m + 1)] = chunk - hi
        return out.astype(BF16)

    wlin = np.concatenate(
        [lin_W[:, 0:128].T, lin_W[:, 128:256].T], axis=1
    ).astype(BF16)
    bl = np.zeros((128, 128), np.float32)
    hi = lin_b.astype(BF16).astype(np.float32)
    bl[0, :] = hi
    bl[1, :] = lin_b - hi
    e0 = np.zeros((128, LB), np.float32)
    e0[0, :] = 1.0
    e0[1, :] = 1.0
    return {
        "wih_e": pack_ih(enc_W_ih),
        "whh_e": pack_hh(enc_W_hh),
        "bias_e": pack_bias(enc_b_ih + enc_b_hh),
        "wih_d": pack_ih(dec_W_ih),
        "whh_d": pack_hh(dec_W_hh),
        "bias_d": pack_bias(dec_b_ih + dec_b_hh),
        "wlin": wlin,
        "bias_l": bl.astype(BF16),
        "e0": e0.astype(BF16),
    }


def _run(inputs, t_steps, trace=False):
    from concourse.bass_utils import run_bass_kernel_spmd

    nc = _get(t_steps)
    x = np.asarray(inputs["x"], np.float32)
    shared = _pack_weights(
        np.asarray(inputs["enc_W_ih"], np.float32),
        np.asarray(inputs["enc_W_hh"], np.float32),
        np.asarray(inputs["enc_b_ih"], np.float32),
        np.asarray(inputs["enc_b_hh"], np.float32),
        np.asarray(inputs["dec_W_ih"], np.float32),
        np.asarray(inputs["dec_W_hh"], np.float32),
        np.asarray(inputs["dec_b_ih"], np.float32),
        np.asarray(inputs["dec_b_hh"], np.float32),
        np.asarray(inputs["lin_W"], np.float32),
        np.asarray(inputs["lin_b"], np.float32),
    )
    in_maps = []
    for j in range(NCORES):
        xs = x[LB * j : LB * (j + 1), :t_steps]  # [32, T, 128]
        xT = np.ascontiguousarray(xs.transpose(2, 1, 0)).reshape(128, t_steps * LB)
        m = dict(shared)
        m["xT"] = xT.astype(BF16)
        in_maps.append(m)

    res = run_bass_kernel_spmd(
        nc, in_maps, list(range(NCORES)), trace=trace
    )
    out = np.empty((B, t_steps, I), np.float32)
    for j in range(NCORES):
        o = res.results[j]["out"].reshape(128, t_steps, LB)
        out[LB * j : LB * (j + 1)] = o.transpose(2, 1, 0)
    return out, res


def kernel(**inputs):
    out, _ = _run(inputs, T)
    return out

